# revision 16
# baseline (speedup 1.0000x reference)
"""Trainium2 Bass kernel for nn_MixedLinear (DARTS-style mixed-precision supernet linear).

Reference math (16-term arch-weighted mixture) reduces algebraically to:

  x_mix = C * round(x)                      C = sum(arch_weights)
          [a_scales == 1 and |x| < 7.5, so both activation fake-quant
           branches equal round-half-even(x)]
  w_mix[o,i] = G0(R,Cc)*s0*clip(round(w/s0),-8,7) + G1(R,Cc)*s1*round(w/s1)
          [fake_quant(w * mask) == mask * fake_quant(w); the four (h,it)
           masks collapse into piecewise-constant coefficients over the
           2x2 region grid R = (o >= 3072), Cc = (i >= 768); the 8-bit
           branch's clip never binds for this data]
  out = x_mix @ w_mix^T + beta(R) * bias
      = round(x) @ W_eff^T + b_mix,   W_eff = C * w_mix

Distribution: 4 token-groups x 2 output-halves across the 8 cores
(tg = core % 4 owns tokens [tg*2048, (tg+1)*2048), oh = core // 4 owns
output features [oh*2048, (oh+1)*2048)). Each core builds its W_eff^T
half on-device and computes out^T[2048, 2048] with fp32r matmuls
(full-rate on the PE, ~1e-4 relative error; plain fp32 matmul is 4x
slower on trn2), contracting K=1024 in 8 partition-tiles.

The program is SPMD-uniform: all value/region-dependent coefficients
arrive as small per-core input tensors (per-partition [128,1] scalar
operands), so a single NEFF serves all 8 cores. Host work is limited to
layout (transpose / shard / concat) and deriving ~12 scalar coefficients
from the 16 arch weights.

Rounding on device uses the magic-number trick: fp32 (v + 1.5*2^23) -
1.5*2^23 == round-half-even(v), matching jnp.round exactly.
"""

import numpy as np

import concourse.mybir as mybir
from concourse import bacc, bass_utils
from concourse.tile import TileContext

N_CORES = 8
B, S, I_DIM, O_DIM = 4, 2048, 1024, 4096
T_TOT = B * S
NTG, NOH = 4, 2         # token groups x output halves
T_SH = T_TOT // NTG     # 2048 tokens per core
O_SH = O_DIM // NOH     # 2048 output features per core
NI = I_DIM // 128       # 8 contraction tiles
O_SPAN = 1024           # o-columns per W_eff stage (region boundary 3072 aligns)
NSP = O_SH // O_SPAN    # 2 spans per core
NOT = O_SPAN // 128     # 8 o-tiles per span
TCH = 512               # matmul moving free dim
NTC = T_SH // TCH       # 4 t-chunks
NBT = O_SH // 128       # 16 bias columns per core
MAGIC = 12582912.0      # 1.5 * 2**23
F32 = mybir.dt.float32
F32R = mybir.dt.float32r
AL = mybir.AluOpType
AF = mybir.ActivationFunctionType

_cache: dict = {}
_last_res = None


def _build():
    """Single SPMD NEFF for all cores; coefficients come in via ptab/bsc."""
    nc = bacc.Bacc("TRN2", target_bir_lowering=False)
    x_t = nc.dram_tensor("x_t", [I_DIM, T_SH], F32, kind="ExternalInput")
    w_t = nc.dram_tensor("w_t", [I_DIM, O_SH], F32, kind="ExternalInput")
    b_pt = nc.dram_tensor("b_pt", [128, NBT], F32, kind="ExternalInput")
    bsc = nc.dram_tensor("bsc", [128, NBT], F32, kind="ExternalInput")
    # ptab cols: 0 inv_s0 | 1 inv_s1 | 2+2*blk+Cc q0 | 10+2*blk+Cc q1
    ptab = nc.dram_tensor("ptab", [128, 18], F32, kind="ExternalInput")
    out_t = nc.dram_tensor("out_t", [O_SH, T_SH], F32, kind="ExternalOutput")

    with TileContext(nc) as tc:
        with (
            tc.tile_pool(name="px", bufs=1) as px,
            tc.tile_pool(name="pstage", bufs=3) as pstage,
            tc.tile_pool(name="ptmp", bufs=3) as ptmp,
            tc.tile_pool(name="pwe", bufs=4) as pwe,
            tc.tile_pool(name="pout", bufs=6) as pout,
            tc.tile_pool(name="psum", bufs=8, space="PSUM") as psum,
        ):
            pt = px.tile([128, 18], F32, tag="pt")
            nc.sync.dma_start(out=pt, in_=ptab[:, :])
            bt = pstage.tile([128, NBT], F32, tag="bt")
            nc.sync.dma_start(out=bt, in_=b_pt[:, :])
            bc = pstage.tile([128, NBT], F32, tag="bc")
            nc.sync.dma_start(out=bc, in_=bsc[:, :])
            bs = px.tile([128, NBT], F32, tag="bs")
            nc.vector.tensor_tensor(out=bs, in0=bt, in1=bc, op=AL.mult)

            # xq[i] = round(x^T tile); integers, exact in fp32r. The fp32r
            # matmul input must be produced by a compute op (not DMA), so
            # stage raw x and round into the resident fp32r tiles. Loaded in
            # t-chunk order so the first matmul chains can start early.
            xq = []
            for i in range(NI):
                q = px.tile([128, T_SH], F32R, tag=f"xq{i}")
                xq.append(q)

            def load_x_chunk(t, eng):
                for i in range(NI):
                    xr = pstage.tile([128, TCH], F32, tag="xr")
                    nc.sync.dma_start(
                        out=xr,
                        in_=x_t[128 * i : 128 * (i + 1), TCH * t : TCH * (t + 1)],
                    )
                    eng.tensor_scalar(
                        xq[i][:, TCH * t : TCH * (t + 1)],
                        xr, MAGIC, MAGIC, AL.add, AL.subtract,
                    )

            ncopy = 0
            NBLK = O_SH // 512   # 4 weight blocks of 512 o-columns
            for blk in range(NBLK):
                wes = []
                for i in range(NI):
                    cc = 1 if i * 128 >= 768 else 0
                    q0ap = pt[:, 2 + 2 * blk + cc : 3 + 2 * blk + cc]
                    q1ap = pt[:, 10 + 2 * blk + cc : 11 + 2 * blk + cc]
                    wr = pstage.tile([128, 512], F32, tag="wr")
                    nc.sync.dma_start(
                        out=wr,
                        in_=w_t[128 * i : 128 * (i + 1), blk * 512 : (blk + 1) * 512],
                    )
                    # t0/t1 = round(w/s{0,1}) + M  (ACT affine fma + magic)
                    t0 = ptmp.tile([128, 512], F32, tag="t0")
                    nc.scalar.activation(t0, wr, AF.Copy, bias=MAGIC, scale=pt[:, 0:1])
                    t1 = ptmp.tile([128, 512], F32, tag="t1")
                    nc.scalar.activation(t1, wr, AF.Copy, bias=MAGIC, scale=pt[:, 1:2])
                    # 4-bit clip in shifted domain (in-place), exact -M, scale
                    nc.vector.tensor_scalar(t0, t0, MAGIC - 8.0, MAGIC + 7.0, AL.max, AL.min)
                    p2 = ptmp.tile([128, 512], F32, tag="p2")
                    nc.vector.tensor_scalar(p2, t0, -MAGIC, q0ap, AL.add, AL.mult)
                    nc.vector.tensor_scalar(t1, t1, -MAGIC, q1ap, AL.add, AL.mult)
                    we = pwe.tile([128, 512], F32R, tag=f"we{i}")
                    nc.vector.tensor_tensor(out=we, in0=p2, in1=t1, op=AL.add)
                    wes.append(we)

                if blk == 0:
                    # x chunk 0 right after the first W block's prep chain (the
                    # longer pole); remaining chunks follow so the first matmul
                    # chains are not queued behind 8 MB of x DMA
                    for t in range(NTC):
                        load_x_chunk(t, nc.vector)

                for t in range(NTC):
                    for ot in range(4):
                        og = blk * 4 + ot  # o-tile index within this core
                        ps = psum.tile([128, TCH], F32, tag="ps")
                        for i in range(NI):
                            nc.tensor.matmul(
                                ps,
                                wes[i][:, 128 * ot : 128 * (ot + 1)],
                                xq[i][:, TCH * t : TCH * (t + 1)],
                                start=(i == 0),
                                stop=(i == NI - 1),
                            )
                        ob = pout.tile([128, TCH], F32, tag="ob")
                        nc.scalar.activation(
                            ob, ps, AF.Identity, bias=bs[:, og : og + 1], scale=1.0
                        )
                        ncopy += 1
                        nc.sync.dma_start(
                            out=out_t[og * 128 : (og + 1) * 128, TCH * t : TCH * (t + 1)],
                            in_=ob,
                        )
    nc.compile()
    return nc


def _derive(arch_weights, w_scales):
    aw = np.asarray(arch_weights, dtype=np.float64)
    S4 = aw.reshape(2, 2, 2, 2)  # [h_idx, it_idx, m, n]
    C = float(aw.sum())
    s0 = float(np.asarray(w_scales)[0])  # 4-bit scale
    s1 = float(np.asarray(w_scales)[1])  # 8-bit scale
    Ssum = S4.sum(axis=2)  # [h, it, n]
    G = np.zeros((2, 2, 2))  # [n, R, Cc]
    for n in (0, 1):
        for R in (0, 1):
            its = (0, 1) if R == 0 else (1,)
            for Cc in (0, 1):
                hs = (0, 1) if Cc == 0 else (1,)
                G[n, R, Cc] = sum(Ssum[h, it, n] for it in its for h in hs)
    q0 = (C * G[0] * s0).astype(np.float32)  # [R][Cc]
    q1 = (C * G[1] * s1).astype(np.float32)
    beta0 = np.float32(C)
    beta1 = np.float32(S4[:, 1].sum())
    inv_s0 = np.float32(1.0 / s0)
    inv_s1 = np.float32(1.0 / s1)
    return inv_s0, inv_s1, q0, q1, beta0, beta1, s0, s1


def _fallback(x, arch_weights, weight, bias, a_scales, w_scales):
    """Exact numpy replica of the reference (guard path; not used for the
    shipped input distribution)."""
    aw = np.asarray(arch_weights, np.float32)
    x = np.asarray(x, np.float32)
    w = np.asarray(weight, np.float32)
    b = np.asarray(bias, np.float32)
    a_s = np.asarray(a_scales, np.float32)
    w_s = np.asarray(w_scales, np.float32)
    rows = np.arange(O_DIM)[:, None]
    cols = np.arange(I_DIM)[None, :]

    def fq(v, scale, bit):
        qn, qp = -(2.0 ** (bit - 1)), 2.0 ** (bit - 1) - 1
        return (np.round(np.clip(v / scale, qn, qp)) * scale).astype(np.float32)

    x_mix = np.zeros_like(x)
    w_mix = np.zeros_like(w)
    b_mix = np.zeros_like(b)
    k = 0
    for h in (768, 1024):
        for it in (3072, 4096):
            mask = ((rows < it) & (cols < h)).astype(np.float32)
            w_pad = w * mask
            b_pad = b * (rows[:, 0] < it).astype(np.float32)
            for m, ab in enumerate((4, 8)):
                for n, wb in enumerate((4, 8)):
                    wk = aw[k]
                    x_mix = x_mix + wk * fq(x, a_s[m], ab)
                    w_mix = w_mix + wk * fq(w_pad, w_s[n], wb)
                    b_mix = b_mix + wk * b_pad
                    k += 1
    return (
        np.einsum("bsi,oi->bso", x_mix, w_mix, optimize=True) + b_mix
    ).astype(np.float32)


def _run(inputs, trace=False):
    global _last_res
    x = np.ascontiguousarray(np.asarray(inputs["x"], np.float32))
    arch_weights = np.asarray(inputs["arch_weights"], np.float32)
    weight = np.ascontiguousarray(np.asarray(inputs["weight"], np.float32))
    bias = np.ascontiguousarray(np.asarray(inputs["bias"], np.float32))
    a_scales = np.asarray(inputs["a_scales"], np.float32)
    w_scales = np.asarray(inputs["w_scales"], np.float32)

    inv_s0, inv_s1, q0, q1, beta0, beta1, s0, s1 = _derive(arch_weights, w_scales)

    # fast-path validity (always true for the shipped input distribution)
    if not (
        np.all(np.abs(a_scales - 1.0) == 0.0)
        and float(np.abs(x).max()) < 7.49
        and float(np.abs(weight).max()) / s1 < 126.9
    ):
        return _fallback(x, arch_weights, weight, bias, a_scales, w_scales), None

    if "nc" not in _cache:
        _cache["nc"] = _build()
    nc = _cache["nc"]

    x2 = x.reshape(T_TOT, I_DIM)
    wt = weight.T  # [I_DIM, O_DIM] view
    x_sh = [np.ascontiguousarray(x2[g * T_SH : (g + 1) * T_SH].T) for g in range(NTG)]
    w_sh = [np.ascontiguousarray(wt[:, h * O_SH : (h + 1) * O_SH]) for h in range(NOH)]
    in_maps = []
    for j in range(N_CORES):
        tg, oh = j % NTG, j // NTG
        b_pt = np.ascontiguousarray(
            bias[oh * O_SH : (oh + 1) * O_SH].reshape(NBT, 128).T
        )
        # beta selector per bias column: global o-tile index >= 24 -> region 1
        sel = np.where(np.arange(NBT) + oh * NBT >= 24, beta1, beta0).astype(np.float32)
        bsc = np.ascontiguousarray(np.broadcast_to(sel, (128, NBT)))
        # region of 512-col weight block blk: R = 1 iff oh == 1 and blk >= 2
        prow = np.zeros(18, np.float32)
        prow[0], prow[1] = inv_s0, inv_s1
        for blk in range(4):
            R = 1 if (oh == 1 and blk >= 2) else 0
            for cc in range(2):
                prow[2 + 2 * blk + cc] = q0[R][cc]
                prow[10 + 2 * blk + cc] = q1[R][cc]
        ptab = np.ascontiguousarray(np.broadcast_to(prow, (128, 18)))
        in_maps.append(
            {"x_t": x_sh[tg], "w_t": w_sh[oh], "b_pt": b_pt, "bsc": bsc, "ptab": ptab}
        )

    res = bass_utils.run_bass_kernel_spmd(
        nc, in_maps, core_ids=list(range(N_CORES)), trace=trace
    )
    _last_res = res
    out = np.empty((T_TOT, O_DIM), np.float32)
    for j in range(N_CORES):
        tg, oh = j % NTG, j // NTG
        out[tg * T_SH : (tg + 1) * T_SH, oh * O_SH : (oh + 1) * O_SH] = res.results[j][
            "out_t"
        ].T
    return out.reshape(B, S, O_DIM), res.exec_time_ns


def kernel(**inputs):
    out, _ = _run(inputs, trace=False)
    return out
